# revision 18
# baseline (speedup 1.0000x reference)
"""Trainium2 Bass kernel for the pairwise-decoder (dense_mlp) problem.

Computes, for s1=16, s2=512 grid of (i,j) pairs:
  out_notes = zV @ zW.T + bu @ bp.T + bu_ @ bp_.T                    [16, 512]
  feats[i,j] = [zV[i], zW[j]];  theta = relu(feats@W1+b1)@W2+b2
  p = softmax(theta); logits = p@W3+b3
  bn = (logits - mean0) * rsqrt(var0 + eps)   (batch stats over all 8192 rows)
  out_text = softmax(bn, axis=-1)                                    [8192, 8000]

Sharding: data-parallel over s1 (2 rows of zV per core -> 1024 grid rows/core).
BN batch stats use the Gram-matrix identity:
    sum_r logits[r,c]   = (pbar . w_c)          with pbar = sum_r p_r
    sum_r logits[r,c]^2 = w_c^T S w_c (+b3 terms), S = sum_r p_r p_r^T
so only S_ext = [p|1]^T [p|1]  (51x51) is all-reduced. Per-column stats math
is sharded over vocab (1000 cols/core via a host-sharded W3 slice), then
rstd / shift rows are all-gathered (b3 cancels exactly: BN output is
invariant to the b3 shift, and b3-mean = -(pbar@W3)/n). rstd and the shift
are folded into the weight matrix: W3s_ext = [W3*rstd ; -(u/n)*rstd],
pT_ext = [p^T ; 1], making the whole BN one K=51 float32r matmul; exp
row-sums come free from the scalar engine's accum_out, so the final softmax
costs one tensor_scalar multiply before the output DMA.
"""

import os
import numpy as np

# The trace path needs antenv.axon_hooks, absent in this container — make sure
# a stray BASS_TRACE in the environment can't break the grading run.
os.environ.setdefault("BASS_NEVER_TRACE", "1")

S1, S2 = 16, 512
INT_DIM, ANOTHER, TOPICS, VOCAB = 50, 80, 50, 8000
N_CORES = 8
SPC = S1 // N_CORES          # s1 rows per core (2)
RPC = SPC * S2               # grid rows per core (1024)
NTOT = S1 * S2               # 8192
VSL = VOCAB // N_CORES       # vocab stats slice per core (1000)
BN_EPS = 1e-5
KE = TOPICS + 1              # 51 (p rows + ones row)

_CACHE = {}

# env knobs for experiments
F32R_BIG = os.environ.get("NNK_F32R", "1") == "1"      # float32r for the big p@W3 matmul
TIMELINE_STUB = os.environ.get("NNK_TIMELINE_STUB", "0") == "1"  # single-core, no collectives
DEBUG_OUT = os.environ.get("NNK_DEBUG", "0") == "1"  # extra intermediate outputs


def _build():
    import concourse.bass as bass
    import concourse.bacc as bacc
    import concourse.tile as tile
    import concourse.mybir as mybir
    from concourse.bass_utils import axon_active

    dt = mybir.dt
    f32 = dt.float32
    AF = mybir.ActivationFunctionType
    ALU = mybir.AluOpType
    AX = mybir.AxisListType

    nc = bacc.Bacc(
        "TRN2",
        target_bir_lowering=False,
        debug=False,
        enable_asserts=False,
        num_devices=1 if TIMELINE_STUB else N_CORES,
    )

    # ---- I/O --------------------------------------------------------------
    P50W = 2 * ANOTHER + SPC + VSL + 1  # w1t|w1b|zvT|w3sl|b2
    pack50 = nc.dram_tensor("pack50", [INT_DIM, P50W], f32, kind="ExternalInput")
    pack80 = nc.dram_tensor("pack80", [ANOTHER, TOPICS + 1], f32, kind="ExternalInput")
    zwT = nc.dram_tensor("zwT", [INT_DIM, S2], f32, kind="ExternalInput")
    notes_lhsT = nc.dram_tensor("notes_lhsT", [INT_DIM + 2, S1], f32, kind="ExternalInput")
    notes_rhs2 = nc.dram_tensor("notes_rhs2", [2, S2], f32, kind="ExternalInput")
    w3 = nc.dram_tensor("w3", [TOPICS, VOCAB], f32, kind="ExternalInput")

    notes_out = nc.dram_tensor("notes", [S1, S2], f32, kind="ExternalOutput")
    text_out = nc.dram_tensor("text_sh", [RPC, VOCAB], f32, kind="ExternalOutput")
    if DEBUG_OUT:
        dbg_sg = nc.dram_tensor("dbg_sg", [KE, KE], f32, kind="ExternalOutput")
        dbg_rstd = nc.dram_tensor("dbg_rstd", [1, VOCAB], f32, kind="ExternalOutput")
        dbg_w3s = nc.dram_tensor("dbg_w3s", [KE, VOCAB], f32, kind="ExternalOutput")
        dbg_pt = nc.dram_tensor("dbg_pt", [KE, RPC], f32, kind="ExternalOutput")
        dbg_stats = nc.dram_tensor("dbg_stats", [4, VSL], f32, kind="ExternalOutput")

    ident_np = np.eye(128, dtype=np.float32)
    ident_dram = nc.inline_tensor(ident_np, name="ident128")

    big_dt = dt.float32r if F32R_BIG else f32

    def as_f32(ap):
        return ap.bitcast(f32) if big_dt != f32 else ap

    with tile.TileContext(nc) as tc:
      with tc.tile_pool(name="persist", bufs=1) as persist:
        with (
            tc.tile_pool(name="const", bufs=1) as constp,
            tc.tile_pool(name="psA", bufs=2, space=bass.MemorySpace.PSUM) as psA,
            tc.tile_pool(name="dram", bufs=1, space=bass.MemorySpace.DRAM) as dramp,
        ):
            # ---- constant loads ------------------------------------------
            ident = constp.tile([128, 128], f32)
            nc.sync.dma_start(ident[:], ident_dram[:])
            ones_row = constp.tile([1, KE], f32)       # [1,51] of ones
            nc.gpsimd.memset(ones_row[:], 1.0)
            ones_col = constp.tile([TOPICS, 1], f32)   # [50,1] of ones
            nc.gpsimd.memset(ones_col[:], 1.0)
            ones_b = constp.tile([1, S2], f32)         # [1,512] of ones
            nc.gpsimd.memset(ones_b[:], 1.0)

            p50_sb = constp.tile([INT_DIM, P50W], f32)
            nc.sync.dma_start(p50_sb[:], pack50[:])
            p80_sb = constp.tile([ANOTHER, TOPICS + 1], f32)
            nc.sync.dma_start(p80_sb[:], pack80[:])
            w1t_sb = p50_sb[:, 0:ANOTHER]
            w1b_sb = p50_sb[:, ANOTHER : 2 * ANOTHER]
            zvT_sb = p50_sb[:, 2 * ANOTHER : 2 * ANOTHER + SPC]
            w3sl_sb = p50_sb[:, 2 * ANOTHER + SPC : 2 * ANOTHER + SPC + VSL]
            b2_sb = p50_sb[:, P50W - 1 : P50W]
            w2_sb = p80_sb[:, 0:TOPICS]
            b1_sb = p80_sb[:, TOPICS : TOPICS + 1]

            # rhs for notes: [52,512] = [zW^T ; bp^T ; bp_^T]
            rhs_notes = constp.tile([INT_DIM + 2, S2], f32)
            nc.sync.dma_start(rhs_notes[0:INT_DIM, :], zwT[:])
            nc.sync.dma_start(rhs_notes[INT_DIM : INT_DIM + 2, :], notes_rhs2[:])
            lhsT_notes = constp.tile([INT_DIM + 2, S1], f32)
            nc.sync.dma_start(lhsT_notes[:], notes_lhsT[:])

            # big weights (row 50 = BN bias row, filled after AllGather)
            w3b = constp.tile([KE, VOCAB], f32)
            nc.sync.dma_start(w3b[0:TOPICS, :], w3[:])
            w3s = persist.tile([KE, VOCAB], big_dt)

            # ---- out_notes: one K=52 matmul ------------------------------
            notes_ps = psA.tile([S1, S2], f32, tag="ps")
            nc.tensor.matmul(notes_ps[:], lhsT_notes[:], rhs_notes[:])
            notes_sb = constp.tile([S1, S2], f32)
            nc.vector.tensor_copy(notes_sb[:], notes_ps[:])
            nc.sync.dma_start(notes_out[:], notes_sb[:])

            # ---- stage 1: p^T per i-block --------------------------------
            # hBT = W1_bot^T @ zW^T  (shared across i)
            hbt_ps = psA.tile([ANOTHER, S2], f32, tag="ps")
            nc.tensor.matmul(hbt_ps[:], w1b_sb, rhs_notes[0:INT_DIM, :])
            hbt_sb = constp.tile([ANOTHER, S2], f32)
            nc.vector.tensor_copy(hbt_sb[:], hbt_ps[:])
            # aT = W1_top^T @ zV^T : [80, SPC]
            at_ps = psA.tile([ANOTHER, SPC], f32, tag="ps")
            nc.tensor.matmul(at_ps[:], w1t_sb, zvT_sb)
            at_sb = constp.tile([ANOTHER, SPC], f32)
            nc.vector.tensor_copy(at_sb[:], at_ps[:])

            pt_ext = []
            for i in range(SPC):
                bias_i = constp.tile([ANOTHER, 1], f32, tag=f"bias{i}")
                nc.vector.tensor_tensor(
                    bias_i[:], at_sb[:, i : i + 1], b1_sb, ALU.add
                )
                ht_i = constp.tile([ANOTHER, S2], f32, tag=f"ht{i}")
                nc.scalar.activation(ht_i[:], hbt_sb[:], AF.Relu, bias=bias_i[:])
                th_ps = psA.tile([TOPICS, S2], f32, tag="ps")
                nc.tensor.matmul(th_ps[:], w2_sb, ht_i[:])
                et_i = constp.tile([TOPICS, S2], f32, tag=f"et{i}")
                nc.scalar.activation(et_i[:], th_ps[:], AF.Exp, bias=b2_sb)
                srow_ps = psA.tile([1, S2], f32, tag="ps")
                nc.tensor.matmul(srow_ps[:], ones_col[:], et_i[:])
                recip_i = constp.tile([1, S2], f32, tag=f"rc{i}")
                nc.vector.reciprocal(recip_i[:], srow_ps[:])
                bc_ps = psA.tile([TOPICS, S2], f32, tag="ps")
                nc.tensor.matmul(bc_ps[:], ones_row[0:1, 0:TOPICS], recip_i[:])
                pt_i = persist.tile([KE, S2], big_dt, tag=f"pt{i}")
                nc.vector.tensor_tensor(pt_i[0:TOPICS, :], et_i[:], bc_ps[:], ALU.mult)
                # ones row via SBUF->SBUF DMA (gpsimd casts f32->f32r; memset
                # can't target partition 50 / f32r directly)
                nc.gpsimd.dma_start(pt_i[TOPICS:KE, :], ones_b[:])
                pt_ext.append(pt_i)

            # ---- S_ext = sum_chunks (pT chunk)^T-gram --------------------
            nchunk = SPC * S2 // 128  # 8
            pch = []
            for c in range(nchunk):
                i, rb = divmod(c, S2 // 128)
                tp_ps = psA.tile([128, KE], f32, tag="ps")
                nc.tensor.transpose(
                    tp_ps[:],
                    as_f32(pt_ext[i][:, rb * 128 : (rb + 1) * 128]),
                    ident[0:KE, 0:KE],
                )
                p_sb = constp.tile([128, KE], f32, tag=f"pch{c}")
                nc.vector.tensor_copy(p_sb[:], tp_ps[:])
                pch.append(p_sb)
            sext_ps = psA.tile([KE, KE], f32, tag="ps")
            for c in range(nchunk):
                nc.tensor.matmul(
                    sext_ps[:], pch[c][:], pch[c][:],
                    start=(c == 0), stop=(c == nchunk - 1),
                )
            sext_sb = constp.tile([KE, KE], f32)
            nc.vector.tensor_copy(sext_sb[:], sext_ps[:])

            # ---- AllReduce S_ext ----------------------------------------
            ar_in = dramp.tile([KE, KE], f32)
            ar_out = dramp.tile([KE, KE], f32)
            nc.sync.dma_start(ar_in[:], sext_sb[:])
            if TIMELINE_STUB:
                nc.sync.dma_start(ar_out[:], ar_in[:])
            else:
                nc.gpsimd.collective_compute(
                    "AllReduce", ALU.add,
                    replica_groups=[list(range(N_CORES))],
                    ins=[ar_in.opt()], outs=[ar_out.opt()],
                )
            sg = constp.tile([KE, KE], f32)
            nc.sync.dma_start(sg[:], ar_out[:])

            # ---- per-core vocab-slice stats ------------------------------
            inv_n = 1.0 / float(NTOT)
            pbar_n = constp.tile([TOPICS, 1], f32)
            nc.vector.tensor_scalar_mul(pbar_n[:], sg[0:TOPICS, TOPICS : TOPICS + 1], -inv_n)

            t_sb = constp.tile([TOPICS, VSL], f32)
            for h in range(2):
                sl = slice(h * 500, (h + 1) * 500)
                t_ps = psA.tile([TOPICS, 500], f32, tag="ps")
                nc.tensor.matmul(t_ps[:], sg[0:TOPICS, 0:TOPICS], w3sl_sb[:, sl])
                nc.vector.tensor_copy(t_sb[:, sl], t_ps[:])
            prod = constp.tile([TOPICS, VSL], f32)
            nc.vector.tensor_tensor(prod[:], t_sb[:], w3sl_sb, ALU.mult)
            # neg_m = -(u/n); BN is invariant to the b3 shift, so b3 never
            # appears: bias row = b3 - mean = neg_m, var = qd/n - (u/n)^2.
            neg_m = constp.tile([1, VSL], f32)
            msq = constp.tile([1, VSL], f32)
            for h in range(2):
                sl = slice(h * 500, (h + 1) * 500)
                qd_ps = psA.tile([1, 500], f32, tag="ps")
                mean_ps = psA.tile([1, 500], f32, tag="ps")
                nc.tensor.matmul(qd_ps[:], ones_col[:], prod[:, sl])
                nc.tensor.matmul(mean_ps[:], pbar_n[:], w3sl_sb[:, sl])
                nc.vector.tensor_copy(neg_m[:, sl], mean_ps[:])
                nc.vector.tensor_scalar_mul(msq[:, sl], qd_ps[:], inv_n)
            var = constp.tile([1, VSL], f32)
            nc.vector.tensor_tensor(var[:], neg_m[:], neg_m[:], ALU.mult)
            nc.vector.tensor_tensor(var[:], msq[:], var[:], ALU.subtract)
            # rstd = exp(-0.5*ln(var+eps))  (avoids low-precision Sqrt table)
            nc.vector.tensor_scalar_add(var[:], var[:], BN_EPS)
            lnv = constp.tile([1, VSL], f32)
            nc.scalar.activation(lnv[:], var[:], AF.Ln)
            rstd_sl = constp.tile([1, VSL], f32)
            nc.scalar.activation(rstd_sl[:], lnv[:], AF.Exp, scale=-0.5)
            biasr_sl = neg_m  # raw shift b3-mean == -(u/n)

            # ---- AllGather rstd/bias slices ------------------------------
            ag_in = dramp.tile([2, VSL], f32)
            ag_out = dramp.tile([2 * N_CORES, VSL], f32)
            nc.sync.dma_start(ag_in[0:1, :], rstd_sl[:])
            nc.sync.dma_start(ag_in[1:2, :], biasr_sl[:])
            if TIMELINE_STUB:
                for k in range(N_CORES):
                    nc.sync.dma_start(ag_out[2 * k : 2 * k + 2, :], ag_in[:])
            else:
                nc.gpsimd.collective_compute(
                    "AllGather", ALU.bypass,
                    replica_groups=[list(range(N_CORES))],
                    ins=[ag_in.opt()], outs=[ag_out.opt()],
                )
            # bias row: rows 1::2 of ag_out, one strided DMA
            ag_r = ag_out[:].rearrange("(k two) v -> two k v", two=2)
            nc.sync.dma_start(
                w3b[TOPICS:KE, :].rearrange("p (k v) -> p k v", k=N_CORES), ag_r[1]
            )
            # rstd broadcast to all 51 partitions straight from DRAM
            rbcast = constp.tile([KE, VOCAB], f32)
            nc.gpsimd.dma_start(
                rbcast[:].rearrange("p (k v) -> p k v", k=N_CORES),
                ag_r[0:1].to_broadcast([KE, N_CORES, VSL]),
            )

            if DEBUG_OUT:
                nc.sync.dma_start(dbg_sg[:], sg[:])
                nc.sync.dma_start(dbg_stats[0:1, :], neg_m[:])
                nc.sync.dma_start(dbg_stats[1:2, :], var[:])
                nc.sync.dma_start(dbg_stats[2:3, :], rstd_sl[:])
                nc.sync.dma_start(dbg_stats[3:4, :], biasr_sl[:])

            # ---- W3s = W3b * rstd (chunked so the big stage can start early)
            for c in range(16):
                sl = slice(c * 500, (c + 1) * 500)
                nc.vector.tensor_tensor(w3s[:, sl], w3b[:, sl], rbcast[:, sl], ALU.mult)

            if DEBUG_OUT:
                nc.sync.dma_start(dbg_rstd[:], rbcast[0:1, :])
                nc.sync.dma_start(dbg_w3s[:], as_f32(w3s[:]))
                for i in range(SPC):
                    nc.sync.dma_start(
                        dbg_pt[:, i * S2 : (i + 1) * S2], as_f32(pt_ext[i][:])
                    )

        # ---- big stage: bn-matmul + exp + row-softmax -------------------
        with (
            tc.tile_pool(name="mm", bufs=2, space=bass.MemorySpace.PSUM) as mmp,
            tc.tile_pool(name="exp", bufs=3) as expp,
            tc.tile_pool(name="small", bufs=3) as smallp,
        ):
            nblk = RPC // 128  # 8
            for blk in range(nblk):
                i, rb = divmod(blk, S2 // 128)
                lhs = pt_ext[i][:, rb * 128 : (rb + 1) * 128]
                exp_t = expp.tile([128, VOCAB], f32, tag="exp")
                racc = smallp.tile([128, 4], f32, tag="racc")
                for g in range(4):
                    g0 = g * 2048
                    gw = min(2048, VOCAB - g0)  # 2048,2048,2048,1856
                    ps = mmp.tile([128, 2048], f32, tag="mm")
                    off = 0
                    while off < gw:
                        tw = min(512, gw - off)
                        nc.tensor.matmul(
                            ps[:, off : off + tw], lhs, w3s[:, g0 + off : g0 + off + tw],
                        )
                        off += tw
                    nc.scalar.activation(
                        exp_t[:, g0 : g0 + gw], ps[:, 0:gw], AF.Exp,
                        accum_out=racc[:, g : g + 1],
                    )
                rsum = smallp.tile([128, 1], f32, tag="rsum")
                nc.vector.reduce_sum(rsum[:], racc[:], axis=AX.X)
                rrec = smallp.tile([128, 1], f32, tag="rrec")
                nc.vector.reciprocal(rrec[:], rsum[:])
                for h in range(4):
                    sl = slice(h * 2000, (h + 1) * 2000)
                    nc.vector.tensor_scalar_mul(exp_t[:, sl], exp_t[:, sl], rrec[:])
                    nc.sync.dma_start(
                        text_out[blk * 128 : (blk + 1) * 128, sl], exp_t[:, sl]
                    )

    nc.compile()
    return nc


def _in_maps(zV, zW, bu, bp, bu_, bp_, W1, b1, W2, b2, W3, b3):
    f = lambda x: np.ascontiguousarray(np.asarray(x, dtype=np.float32))
    maps = []
    notes_lhsT = f(np.concatenate([zV.T, bu.T, bu_.T], axis=0))
    notes_rhs2 = f(np.concatenate([bp.T, bp_.T], axis=0))
    zwT = f(np.asarray(zW).T)
    common = {
        "zwT": zwT,
        "notes_lhsT": notes_lhsT,
        "notes_rhs2": notes_rhs2,
        "pack80": f(np.concatenate([np.asarray(W2), np.asarray(b1).reshape(ANOTHER, 1)], axis=1)),
        "w3": f(W3),
    }
    W1t = np.asarray(W1)[:INT_DIM]
    W1b = np.asarray(W1)[INT_DIM:]
    b2c = np.asarray(b2).reshape(TOPICS, 1)
    for k in range(N_CORES):
        m = dict(common)
        m["pack50"] = f(np.concatenate([
            W1t, W1b,
            np.asarray(zV)[k * SPC : (k + 1) * SPC].T,
            np.asarray(W3)[:, k * VSL : (k + 1) * VSL],
            b2c,
        ], axis=1))
        maps.append(m)
    return maps


LAST_RESULTS = None


def kernel(zV, zW, bu, bp, bu_, bp_, W1, b1, W2, b2, W3, b3):
    global LAST_RESULTS
    from concourse.bass_utils import run_bass_kernel_spmd

    key = "nc"
    if key not in _CACHE:
        _CACHE[key] = _build()
    nc = _CACHE[key]

    maps = _in_maps(zV, zW, bu, bp, bu_, bp_, W1, b1, W2, b2, W3, b3)
    res = run_bass_kernel_spmd(nc, maps, list(range(N_CORES)))
    LAST_RESULTS = res
    out_notes = np.asarray(res.results[0]["notes"], dtype=np.float32)
    out_text = np.concatenate(
        [np.asarray(res.results[k]["text_sh"], dtype=np.float32) for k in range(N_CORES)],
        axis=0,
    )
    return (out_notes, out_text)


# revision 24
# speedup vs baseline: 1.2255x; 1.2255x over previous
"""Trainium2 Bass kernel for the pairwise-decoder (dense_mlp) problem.

Computes, for s1=16, s2=512 grid of (i,j) pairs:
  out_notes = zV @ zW.T + bu @ bp.T + bu_ @ bp_.T                    [16, 512]
  feats[i,j] = [zV[i], zW[j]];  theta = relu(feats@W1+b1)@W2+b2
  p = softmax(theta); logits = p@W3+b3
  bn = (logits - mean0) * rsqrt(var0 + eps)   (batch stats over all 8192 rows)
  out_text = softmax(bn, axis=-1)                                    [8192, 8000]

Sharding: data-parallel over s1 (2 rows of zV per core -> 1024 grid rows/core).
BN batch stats use the Gram-matrix identity:
    sum_r logits[r,c]   = (pbar . w_c)          with pbar = sum_r p_r
    sum_r logits[r,c]^2 = w_c^T S w_c (+b3 terms), S = sum_r p_r p_r^T
so only S_ext = [p|1]^T [p|1]  (51x51) is all-reduced. Per-column stats math
is sharded over vocab (1000 cols/core via a host-sharded W3 slice), then
rstd / shift rows are all-gathered (b3 cancels exactly: BN output is
invariant to the b3 shift, and b3-mean = -(pbar@W3)/n). rstd and the shift
are folded into the weight matrix: W3s_ext = [W3*rstd ; -(u/n)*rstd],
pT_ext = [p^T ; 1], making the whole BN one K=51 float32r matmul; exp
row-sums come free from the scalar engine's accum_out, so the final softmax
costs one tensor_scalar multiply before the output DMA.
"""

import os
import numpy as np

# The trace path needs antenv.axon_hooks, absent in this container — make sure
# a stray BASS_TRACE in the environment can't break the grading run.
os.environ.setdefault("BASS_NEVER_TRACE", "1")

S1, S2 = 16, 512
INT_DIM, ANOTHER, TOPICS, VOCAB = 50, 80, 50, 8000
N_CORES = 8
SPC = S1 // N_CORES          # s1 rows per core (2)
RPC = SPC * S2               # grid rows per core (1024)
NTOT = S1 * S2               # 8192
VSL = VOCAB // N_CORES       # vocab stats slice per core (1000)
BN_EPS = 1e-5
KE = TOPICS + 1              # 51 (p rows + ones row)

_CACHE = {}

# env knobs for experiments
F32R_BIG = os.environ.get("NNK_F32R", "1") == "1"      # float32r for the big p@W3 matmul
OUT16 = os.environ.get("NNK_OUT16", "1") == "1"        # bf16 out_text DMA (host upcasts)
TIMELINE_STUB = os.environ.get("NNK_TIMELINE_STUB", "0") == "1"  # single-core, no collectives
DEBUG_OUT = os.environ.get("NNK_DEBUG", "0") == "1"  # extra intermediate outputs


def _build():
    import concourse.bass as bass
    import concourse.bacc as bacc
    import concourse.tile as tile
    import concourse.mybir as mybir
    from concourse.bass_utils import axon_active

    dt = mybir.dt
    f32 = dt.float32
    AF = mybir.ActivationFunctionType
    ALU = mybir.AluOpType
    AX = mybir.AxisListType

    nc = bacc.Bacc(
        "TRN2",
        target_bir_lowering=False,
        debug=False,
        enable_asserts=False,
        num_devices=1 if TIMELINE_STUB else N_CORES,
    )

    # ---- I/O --------------------------------------------------------------
    P50W = 2 * ANOTHER + SPC + VSL + 1  # w1t|w1b|zvT|w3sl|b2
    pack50 = nc.dram_tensor("pack50", [INT_DIM, P50W], f32, kind="ExternalInput")
    pack80 = nc.dram_tensor("pack80", [ANOTHER, TOPICS + 1], f32, kind="ExternalInput")
    zwT = nc.dram_tensor("zwT", [INT_DIM, S2], f32, kind="ExternalInput")
    notes_lhsT = nc.dram_tensor("notes_lhsT", [INT_DIM + 2, S1], f32, kind="ExternalInput")
    notes_rhs2 = nc.dram_tensor("notes_rhs2", [2, S2], f32, kind="ExternalInput")
    w3 = nc.dram_tensor("w3", [TOPICS, VOCAB], f32, kind="ExternalInput")

    notes_out = nc.dram_tensor("notes", [S1, S2], f32, kind="ExternalOutput")
    out_dt = dt.bfloat16 if OUT16 else f32
    text_out = nc.dram_tensor("text_sh", [RPC, VOCAB], out_dt, kind="ExternalOutput")
    if DEBUG_OUT:
        dbg_sg = nc.dram_tensor("dbg_sg", [KE, KE], f32, kind="ExternalOutput")
        dbg_rstd = nc.dram_tensor("dbg_rstd", [1, VOCAB], f32, kind="ExternalOutput")
        dbg_w3s = nc.dram_tensor("dbg_w3s", [KE, VOCAB], f32, kind="ExternalOutput")
        dbg_pt = nc.dram_tensor("dbg_pt", [KE, RPC], f32, kind="ExternalOutput")
        dbg_stats = nc.dram_tensor("dbg_stats", [4, VSL], f32, kind="ExternalOutput")

    ident_np = np.eye(128, dtype=np.float32)
    ident_dram = nc.inline_tensor(ident_np, name="ident128")

    big_dt = dt.float32r if F32R_BIG else f32

    def as_f32(ap):
        return ap.bitcast(f32) if big_dt != f32 else ap

    with tile.TileContext(nc) as tc:
      with tc.tile_pool(name="persist", bufs=1) as persist:
        with (
            tc.tile_pool(name="const", bufs=1) as constp,
            tc.tile_pool(name="psA", bufs=2, space=bass.MemorySpace.PSUM) as psA,
            tc.tile_pool(name="dram", bufs=1, space=bass.MemorySpace.DRAM) as dramp,
        ):
            # ---- constant loads ------------------------------------------
            ident = constp.tile([128, 128], f32)
            nc.sync.dma_start(ident[:], ident_dram[:])
            ones_row = constp.tile([1, KE], f32)       # [1,51] of ones
            nc.gpsimd.memset(ones_row[:], 1.0)
            ones_col = constp.tile([TOPICS, 1], f32)   # [50,1] of ones
            nc.gpsimd.memset(ones_col[:], 1.0)
            ones_b = constp.tile([1, S2], f32)         # [1,512] of ones
            nc.gpsimd.memset(ones_b[:], 1.0)

            p50_sb = constp.tile([INT_DIM, P50W], f32)
            nc.sync.dma_start(p50_sb[:], pack50[:])
            p80_sb = constp.tile([ANOTHER, TOPICS + 1], f32)
            nc.sync.dma_start(p80_sb[:], pack80[:])
            w1t_sb = p50_sb[:, 0:ANOTHER]
            w1b_sb = p50_sb[:, ANOTHER : 2 * ANOTHER]
            zvT_sb = p50_sb[:, 2 * ANOTHER : 2 * ANOTHER + SPC]
            w3sl_sb = p50_sb[:, 2 * ANOTHER + SPC : 2 * ANOTHER + SPC + VSL]
            b2_sb = p50_sb[:, P50W - 1 : P50W]
            w2_sb = p80_sb[:, 0:TOPICS]
            b1_sb = p80_sb[:, TOPICS : TOPICS + 1]

            # rhs for notes: [52,512] = [zW^T ; bp^T ; bp_^T]
            rhs_notes = constp.tile([INT_DIM + 2, S2], f32)
            nc.sync.dma_start(rhs_notes[0:INT_DIM, :], zwT[:])
            nc.sync.dma_start(rhs_notes[INT_DIM : INT_DIM + 2, :], notes_rhs2[:])
            lhsT_notes = constp.tile([INT_DIM + 2, S1], f32)
            nc.sync.dma_start(lhsT_notes[:], notes_lhsT[:])

            # big weights (row 50 = BN bias row, filled after AllGather)
            w3b = constp.tile([KE, VOCAB], f32)
            nc.sync.dma_start(w3b[0:TOPICS, :], w3[:])
            w3s = persist.tile([KE, VOCAB], big_dt)

            # ---- out_notes: one K=52 matmul ------------------------------
            notes_ps = psA.tile([S1, S2], f32, tag="ps")
            nc.tensor.matmul(notes_ps[:], lhsT_notes[:], rhs_notes[:])
            notes_sb = constp.tile([S1, S2], f32)
            nc.vector.tensor_copy(notes_sb[:], notes_ps[:])
            nc.sync.dma_start(notes_out[:], notes_sb[:])

            # ---- stage 1: p^T per i-block --------------------------------
            # hBT = W1_bot^T @ zW^T  (shared across i)
            hbt_ps = psA.tile([ANOTHER, S2], f32, tag="ps")
            nc.tensor.matmul(hbt_ps[:], w1b_sb, rhs_notes[0:INT_DIM, :])
            hbt_sb = constp.tile([ANOTHER, S2], f32)
            nc.vector.tensor_copy(hbt_sb[:], hbt_ps[:])
            # aT = W1_top^T @ zV^T : [80, SPC]
            at_ps = psA.tile([ANOTHER, SPC], f32, tag="ps")
            nc.tensor.matmul(at_ps[:], w1t_sb, zvT_sb)
            at_sb = constp.tile([ANOTHER, SPC], f32)
            nc.vector.tensor_copy(at_sb[:], at_ps[:])

            pt_ext = []
            for i in range(SPC):
                bias_i = constp.tile([ANOTHER, 1], f32, tag=f"bias{i}")
                nc.vector.tensor_tensor(
                    bias_i[:], at_sb[:, i : i + 1], b1_sb, ALU.add
                )
                ht_i = constp.tile([ANOTHER, S2], f32, tag=f"ht{i}")
                nc.scalar.activation(ht_i[:], hbt_sb[:], AF.Relu, bias=bias_i[:])
                th_ps = psA.tile([TOPICS, S2], f32, tag="ps")
                nc.tensor.matmul(th_ps[:], w2_sb, ht_i[:])
                et_i = constp.tile([TOPICS, S2], f32, tag=f"et{i}")
                nc.scalar.activation(et_i[:], th_ps[:], AF.Exp, bias=b2_sb)
                srow_ps = psA.tile([1, S2], f32, tag="ps")
                nc.tensor.matmul(srow_ps[:], ones_col[:], et_i[:])
                recip_i = constp.tile([1, S2], f32, tag=f"rc{i}")
                nc.vector.reciprocal(recip_i[:], srow_ps[:])
                bc_ps = psA.tile([TOPICS, S2], f32, tag="ps")
                nc.tensor.matmul(bc_ps[:], ones_row[0:1, 0:TOPICS], recip_i[:])
                pt_i = persist.tile([KE, S2], big_dt, tag=f"pt{i}")
                nc.vector.tensor_tensor(pt_i[0:TOPICS, :], et_i[:], bc_ps[:], ALU.mult)
                # ones row via SBUF->SBUF DMA (gpsimd casts f32->f32r; memset
                # can't target partition 50 / f32r directly)
                nc.gpsimd.dma_start(pt_i[TOPICS:KE, :], ones_b[:])
                pt_ext.append(pt_i)

            # ---- S_ext = sum_chunks (pT chunk)^T-gram --------------------
            nchunk = SPC * S2 // 128  # 8
            pch = []
            for c in range(nchunk):
                i, rb = divmod(c, S2 // 128)
                tp_ps = psA.tile([128, KE], f32, tag="ps")
                nc.tensor.transpose(
                    tp_ps[:],
                    as_f32(pt_ext[i][:, rb * 128 : (rb + 1) * 128]),
                    ident[0:KE, 0:KE],
                )
                p_sb = constp.tile([128, KE], f32, tag=f"pch{c}")
                nc.vector.tensor_copy(p_sb[:], tp_ps[:])
                pch.append(p_sb)
            sext_ps = psA.tile([KE, KE], f32, tag="ps")
            for c in range(nchunk):
                nc.tensor.matmul(
                    sext_ps[:], pch[c][:], pch[c][:],
                    start=(c == 0), stop=(c == nchunk - 1),
                )
            sext_sb = constp.tile([KE, KE], f32)
            nc.vector.tensor_copy(sext_sb[:], sext_ps[:])

            # ---- AllReduce S_ext ----------------------------------------
            ar_in = dramp.tile([KE, KE], f32)
            ar_out = dramp.tile([KE, KE], f32)
            nc.sync.dma_start(ar_in[:], sext_sb[:])
            if TIMELINE_STUB:
                nc.sync.dma_start(ar_out[:], ar_in[:])
            else:
                nc.gpsimd.collective_compute(
                    "AllReduce", ALU.add,
                    replica_groups=[list(range(N_CORES))],
                    ins=[ar_in.opt()], outs=[ar_out.opt()],
                )
            sg = constp.tile([KE, KE], f32)
            nc.sync.dma_start(sg[:], ar_out[:])

            # ---- per-core vocab-slice stats ------------------------------
            inv_n = 1.0 / float(NTOT)
            pbar_n = constp.tile([TOPICS, 1], f32)
            nc.vector.tensor_scalar_mul(pbar_n[:], sg[0:TOPICS, TOPICS : TOPICS + 1], -inv_n)

            t_sb = constp.tile([TOPICS, VSL], f32)
            for h in range(2):
                sl = slice(h * 500, (h + 1) * 500)
                t_ps = psA.tile([TOPICS, 500], f32, tag="ps")
                nc.tensor.matmul(t_ps[:], sg[0:TOPICS, 0:TOPICS], w3sl_sb[:, sl])
                nc.vector.tensor_copy(t_sb[:, sl], t_ps[:])
            prod = constp.tile([TOPICS, VSL], f32)
            nc.vector.tensor_tensor(prod[:], t_sb[:], w3sl_sb, ALU.mult)
            # neg_m = -(u/n); BN is invariant to the b3 shift, so b3 never
            # appears: bias row = b3 - mean = neg_m, var = qd/n - (u/n)^2.
            neg_m = constp.tile([1, VSL], f32)
            msq = constp.tile([1, VSL], f32)
            m2 = constp.tile([1, VSL], f32)
            for h in range(2):
                sl = slice(h * 500, (h + 1) * 500)
                qd_ps = psA.tile([1, 500], f32, tag="ps")
                mean_ps = psA.tile([1, 500], f32, tag="ps")
                nc.tensor.matmul(qd_ps[:], ones_col[:], prod[:, sl])
                nc.tensor.matmul(mean_ps[:], pbar_n[:], w3sl_sb[:, sl])
                nc.vector.tensor_copy(neg_m[:, sl], mean_ps[:])
                nc.scalar.activation(m2[:, sl], mean_ps[:], AF.Square)
                nc.vector.tensor_scalar_mul(msq[:, sl], qd_ps[:], inv_n)
            var = constp.tile([1, VSL], f32)
            nc.vector.tensor_tensor(var[:], msq[:], m2[:], ALU.subtract)
            # rstd = exp(-0.5*ln(var+eps))  (avoids low-precision Sqrt table)
            nc.vector.tensor_scalar_add(var[:], var[:], BN_EPS)
            lnv = constp.tile([1, VSL], f32)
            nc.scalar.activation(lnv[:], var[:], AF.Ln)
            rstd_sl = constp.tile([1, VSL], f32)
            nc.scalar.activation(rstd_sl[:], lnv[:], AF.Exp, scale=-0.5)
            biasr_sl = neg_m  # raw shift b3-mean == -(u/n)

            # ---- AllGather rstd/bias slices ------------------------------
            ag_in = dramp.tile([2, VSL], f32)
            ag_out = dramp.tile([2 * N_CORES, VSL], f32)
            nc.sync.dma_start(ag_in[0:1, :], rstd_sl[:])
            nc.sync.dma_start(ag_in[1:2, :], biasr_sl[:])
            if TIMELINE_STUB:
                for k in range(N_CORES):
                    nc.sync.dma_start(ag_out[2 * k : 2 * k + 2, :], ag_in[:])
            else:
                nc.gpsimd.collective_compute(
                    "AllGather", ALU.bypass,
                    replica_groups=[list(range(N_CORES))],
                    ins=[ag_in.opt()], outs=[ag_out.opt()],
                )
            # bias row: rows 1::2 of ag_out, one strided DMA
            ag_r = ag_out[:].rearrange("(k two) v -> two k v", two=2)
            nc.sync.dma_start(
                w3b[TOPICS:KE, :].rearrange("p (k v) -> p k v", k=N_CORES), ag_r[1]
            )
            # rstd broadcast to all 51 partitions straight from DRAM,
            # in 2-slice chunks so the W3s scaling can start early
            rbcast = constp.tile([KE, VOCAB], f32)
            for q in range(4):
                ksl = slice(2 * q, 2 * q + 2)
                nc.gpsimd.dma_start(
                    rbcast[:, 2 * q * VSL : (2 * q + 2) * VSL].rearrange(
                        "p (k v) -> p k v", k=2
                    ),
                    ag_r[0:1, ksl].to_broadcast([KE, 2, VSL]),
                )

            if DEBUG_OUT:
                nc.sync.dma_start(dbg_sg[:], sg[:])
                nc.sync.dma_start(dbg_stats[0:1, :], neg_m[:])
                nc.sync.dma_start(dbg_stats[1:2, :], var[:])
                nc.sync.dma_start(dbg_stats[2:3, :], rstd_sl[:])
                nc.sync.dma_start(dbg_stats[3:4, :], biasr_sl[:])

            # ---- W3s = W3b * rstd (chunked so the big stage can start early)
            for c in range(16):
                sl = slice(c * 500, (c + 1) * 500)
                nc.vector.tensor_tensor(w3s[:, sl], w3b[:, sl], rbcast[:, sl], ALU.mult)

            if DEBUG_OUT:
                nc.sync.dma_start(dbg_rstd[:], rbcast[0:1, :])
                nc.sync.dma_start(dbg_w3s[:], as_f32(w3s[:]))
                for i in range(SPC):
                    nc.sync.dma_start(
                        dbg_pt[:, i * S2 : (i + 1) * S2], as_f32(pt_ext[i][:])
                    )

        # ---- big stage: bn-matmul + exp + row-softmax -------------------
        with (
            tc.tile_pool(name="mm", bufs=2, space=bass.MemorySpace.PSUM) as mmp,
            tc.tile_pool(name="exp", bufs=3) as expp,
            tc.tile_pool(name="small", bufs=3) as smallp,
        ):
            nblk = RPC // 128  # 8
            for blk in range(nblk):
                i, rb = divmod(blk, S2 // 128)
                lhs = pt_ext[i][:, rb * 128 : (rb + 1) * 128]
                exp_t = expp.tile([128, VOCAB], f32, tag="exp")
                racc = smallp.tile([128, 4], f32, tag="racc")
                for g in range(4):
                    g0 = g * 2048
                    gw = min(2048, VOCAB - g0)  # 2048,2048,2048,1856
                    ps = mmp.tile([128, 2048], f32, tag="mm")
                    off = 0
                    while off < gw:
                        tw = min(512, gw - off)
                        nc.tensor.matmul(
                            ps[:, off : off + tw], lhs, w3s[:, g0 + off : g0 + off + tw],
                        )
                        off += tw
                    nc.scalar.activation(
                        exp_t[:, g0 : g0 + gw], ps[:, 0:gw], AF.Exp,
                        accum_out=racc[:, g : g + 1],
                    )
                rsum = smallp.tile([128, 1], f32, tag="rsum")
                nc.vector.reduce_sum(rsum[:], racc[:], axis=AX.X)
                rrec = smallp.tile([128, 1], f32, tag="rrec")
                nc.vector.reciprocal(rrec[:], rsum[:])
                if OUT16:
                    out_t = smallp.tile([128, VOCAB], out_dt, tag="out16")
                else:
                    out_t = exp_t
                for h in range(4):
                    sl = slice(h * 2000, (h + 1) * 2000)
                    nc.vector.tensor_scalar_mul(out_t[:, sl], exp_t[:, sl], rrec[:])
                    nc.sync.dma_start(
                        text_out[blk * 128 : (blk + 1) * 128, sl], out_t[:, sl]
                    )

    nc.compile()
    return nc


def _in_maps(zV, zW, bu, bp, bu_, bp_, W1, b1, W2, b2, W3, b3):
    f = lambda x: np.ascontiguousarray(np.asarray(x, dtype=np.float32))
    maps = []
    notes_lhsT = f(np.concatenate([zV.T, bu.T, bu_.T], axis=0))
    notes_rhs2 = f(np.concatenate([bp.T, bp_.T], axis=0))
    zwT = f(np.asarray(zW).T)
    common = {
        "zwT": zwT,
        "notes_lhsT": notes_lhsT,
        "notes_rhs2": notes_rhs2,
        "pack80": f(np.concatenate([np.asarray(W2), np.asarray(b1).reshape(ANOTHER, 1)], axis=1)),
        "w3": f(W3),
    }
    W1t = np.asarray(W1)[:INT_DIM]
    W1b = np.asarray(W1)[INT_DIM:]
    b2c = np.asarray(b2).reshape(TOPICS, 1)
    for k in range(N_CORES):
        m = dict(common)
        m["pack50"] = f(np.concatenate([
            W1t, W1b,
            np.asarray(zV)[k * SPC : (k + 1) * SPC].T,
            np.asarray(W3)[:, k * VSL : (k + 1) * VSL],
            b2c,
        ], axis=1))
        maps.append(m)
    return maps


LAST_RESULTS = None


def kernel(zV, zW, bu, bp, bu_, bp_, W1, b1, W2, b2, W3, b3):
    global LAST_RESULTS
    from concourse.bass_utils import run_bass_kernel_spmd

    key = "nc"
    if key not in _CACHE:
        _CACHE[key] = _build()
    nc = _CACHE[key]

    maps = _in_maps(zV, zW, bu, bp, bu_, bp_, W1, b1, W2, b2, W3, b3)
    res = run_bass_kernel_spmd(nc, maps, list(range(N_CORES)))
    LAST_RESULTS = res
    out_notes = np.asarray(res.results[0]["notes"], dtype=np.float32)
    out_text = np.concatenate(
        [np.asarray(res.results[k]["text_sh"]).astype(np.float32) for k in range(N_CORES)],
        axis=0,
    )
    return (out_notes, out_text)


# revision 25
# speedup vs baseline: 1.2312x; 1.0047x over previous
"""Trainium2 Bass kernel for the pairwise-decoder (dense_mlp) problem.

Computes, for s1=16, s2=512 grid of (i,j) pairs:
  out_notes = zV @ zW.T + bu @ bp.T + bu_ @ bp_.T                    [16, 512]
  feats[i,j] = [zV[i], zW[j]];  theta = relu(feats@W1+b1)@W2+b2
  p = softmax(theta); logits = p@W3+b3
  bn = (logits - mean0) * rsqrt(var0 + eps)   (batch stats over all 8192 rows)
  out_text = softmax(bn, axis=-1)                                    [8192, 8000]

Sharding: data-parallel over s1 (2 rows of zV per core -> 1024 grid rows/core).
BN batch stats use the Gram-matrix identity:
    sum_r logits[r,c]   = (pbar . w_c)          with pbar = sum_r p_r
    sum_r logits[r,c]^2 = w_c^T S w_c (+b3 terms), S = sum_r p_r p_r^T
so only S_ext = [p|1]^T [p|1]  (51x51) is all-reduced. Per-column stats math
is sharded over vocab (1000 cols/core via a host-sharded W3 slice), then
rstd / shift rows are all-gathered (b3 cancels exactly: BN output is
invariant to the b3 shift, and b3-mean = -(pbar@W3)/n). rstd and the shift
are folded into the weight matrix: W3s_ext = [W3*rstd ; -(u/n)*rstd],
pT_ext = [p^T ; 1], making the whole BN one K=51 float32r matmul; exp
row-sums come free from the scalar engine's accum_out, so the final softmax
costs one tensor_scalar multiply before the output DMA.
"""

import os
import numpy as np

# The trace path needs antenv.axon_hooks, absent in this container — make sure
# a stray BASS_TRACE in the environment can't break the grading run.
os.environ.setdefault("BASS_NEVER_TRACE", "1")

S1, S2 = 16, 512
INT_DIM, ANOTHER, TOPICS, VOCAB = 50, 80, 50, 8000
N_CORES = 8
SPC = S1 // N_CORES          # s1 rows per core (2)
RPC = SPC * S2               # grid rows per core (1024)
NTOT = S1 * S2               # 8192
VSL = VOCAB // N_CORES       # vocab stats slice per core (1000)
BN_EPS = 1e-5
KE = TOPICS + 1              # 51 (p rows + ones row)

_CACHE = {}

# env knobs for experiments
F32R_BIG = os.environ.get("NNK_F32R", "1") == "1"      # float32r for the big p@W3 matmul
OUT16 = os.environ.get("NNK_OUT16", "1") == "1"        # bf16 out_text DMA (host upcasts)
TIMELINE_STUB = os.environ.get("NNK_TIMELINE_STUB", "0") == "1"  # single-core, no collectives
DEBUG_OUT = os.environ.get("NNK_DEBUG", "0") == "1"  # extra intermediate outputs


def _build():
    import concourse.bass as bass
    import concourse.bacc as bacc
    import concourse.tile as tile
    import concourse.mybir as mybir
    from concourse.bass_utils import axon_active

    dt = mybir.dt
    f32 = dt.float32
    AF = mybir.ActivationFunctionType
    ALU = mybir.AluOpType
    AX = mybir.AxisListType

    nc = bacc.Bacc(
        "TRN2",
        target_bir_lowering=False,
        debug=False,
        enable_asserts=False,
        num_devices=1 if TIMELINE_STUB else N_CORES,
    )

    # ---- I/O --------------------------------------------------------------
    P50W = 2 * ANOTHER + SPC + VSL + 1  # w1t|w1b|zvT|w3sl|b2
    pack50 = nc.dram_tensor("pack50", [INT_DIM, P50W], f32, kind="ExternalInput")
    pack80 = nc.dram_tensor("pack80", [ANOTHER, TOPICS + 1], f32, kind="ExternalInput")
    zwT = nc.dram_tensor("zwT", [INT_DIM, S2], f32, kind="ExternalInput")
    notes_lhsT = nc.dram_tensor("notes_lhsT", [INT_DIM + 2, S1], f32, kind="ExternalInput")
    notes_rhs2 = nc.dram_tensor("notes_rhs2", [2, S2], f32, kind="ExternalInput")
    w3 = nc.dram_tensor("w3", [TOPICS, VOCAB], f32, kind="ExternalInput")

    notes_out = nc.dram_tensor("notes", [S1, S2], f32, kind="ExternalOutput")
    out_dt = dt.bfloat16 if OUT16 else f32
    text_out = nc.dram_tensor("text_sh", [RPC, VOCAB], out_dt, kind="ExternalOutput")
    if DEBUG_OUT:
        dbg_sg = nc.dram_tensor("dbg_sg", [KE, KE], f32, kind="ExternalOutput")
        dbg_rstd = nc.dram_tensor("dbg_rstd", [1, VOCAB], f32, kind="ExternalOutput")
        dbg_w3s = nc.dram_tensor("dbg_w3s", [KE, VOCAB], f32, kind="ExternalOutput")
        dbg_pt = nc.dram_tensor("dbg_pt", [KE, RPC], f32, kind="ExternalOutput")
        dbg_stats = nc.dram_tensor("dbg_stats", [4, VSL], f32, kind="ExternalOutput")

    ident_np = np.eye(128, dtype=np.float32)
    ident_dram = nc.inline_tensor(ident_np, name="ident128")

    big_dt = dt.float32r if F32R_BIG else f32

    def as_f32(ap):
        return ap.bitcast(f32) if big_dt != f32 else ap

    with tile.TileContext(nc) as tc:
      with tc.tile_pool(name="persist", bufs=1) as persist:
        with (
            tc.tile_pool(name="const", bufs=1) as constp,
            tc.tile_pool(name="psA", bufs=2, space=bass.MemorySpace.PSUM) as psA,
            tc.tile_pool(name="dram", bufs=1, space=bass.MemorySpace.DRAM) as dramp,
        ):
            # ---- constant loads ------------------------------------------
            ident = constp.tile([128, 128], f32)
            nc.sync.dma_start(ident[:], ident_dram[:])
            ones_row = constp.tile([1, KE], f32)       # [1,51] of ones
            nc.gpsimd.memset(ones_row[:], 1.0)
            ones_col = constp.tile([TOPICS, 1], f32)   # [50,1] of ones
            nc.gpsimd.memset(ones_col[:], 1.0)
            ones_b = constp.tile([1, S2], f32)         # [1,512] of ones
            nc.gpsimd.memset(ones_b[:], 1.0)

            p50_sb = constp.tile([INT_DIM, P50W], f32)
            nc.sync.dma_start(p50_sb[:], pack50[:])
            p80_sb = constp.tile([ANOTHER, TOPICS + 1], f32)
            nc.sync.dma_start(p80_sb[:], pack80[:])
            w1t_sb = p50_sb[:, 0:ANOTHER]
            w1b_sb = p50_sb[:, ANOTHER : 2 * ANOTHER]
            zvT_sb = p50_sb[:, 2 * ANOTHER : 2 * ANOTHER + SPC]
            w3sl_sb = p50_sb[:, 2 * ANOTHER + SPC : 2 * ANOTHER + SPC + VSL]
            b2_sb = p50_sb[:, P50W - 1 : P50W]
            w2_sb = p80_sb[:, 0:TOPICS]
            b1_sb = p80_sb[:, TOPICS : TOPICS + 1]

            # rhs for notes: [52,512] = [zW^T ; bp^T ; bp_^T]
            rhs_notes = constp.tile([INT_DIM + 2, S2], f32)
            nc.sync.dma_start(rhs_notes[0:INT_DIM, :], zwT[:])
            nc.sync.dma_start(rhs_notes[INT_DIM : INT_DIM + 2, :], notes_rhs2[:])
            lhsT_notes = constp.tile([INT_DIM + 2, S1], f32)
            nc.sync.dma_start(lhsT_notes[:], notes_lhsT[:])

            # big weights (row 50 = BN bias row, filled after AllGather)
            w3b = constp.tile([KE, VOCAB], f32)
            nc.sync.dma_start(w3b[0:TOPICS, :], w3[:])
            w3s = persist.tile([KE, VOCAB], big_dt)

            # ---- out_notes: one K=52 matmul ------------------------------
            notes_ps = psA.tile([S1, S2], f32, tag="ps")
            nc.tensor.matmul(notes_ps[:], lhsT_notes[:], rhs_notes[:])
            notes_sb = constp.tile([S1, S2], f32)
            nc.vector.tensor_copy(notes_sb[:], notes_ps[:])
            nc.sync.dma_start(notes_out[:], notes_sb[:])

            # ---- stage 1: p^T per i-block --------------------------------
            # hBT = W1_bot^T @ zW^T  (shared across i)
            hbt_ps = psA.tile([ANOTHER, S2], f32, tag="ps")
            nc.tensor.matmul(hbt_ps[:], w1b_sb, rhs_notes[0:INT_DIM, :])
            hbt_sb = constp.tile([ANOTHER, S2], f32)
            nc.vector.tensor_copy(hbt_sb[:], hbt_ps[:])
            # aT = W1_top^T @ zV^T : [80, SPC]
            at_ps = psA.tile([ANOTHER, SPC], f32, tag="ps")
            nc.tensor.matmul(at_ps[:], w1t_sb, zvT_sb)
            at_sb = constp.tile([ANOTHER, SPC], f32)
            nc.vector.tensor_copy(at_sb[:], at_ps[:])

            pt_ext = []
            for i in range(SPC):
                bias_i = constp.tile([ANOTHER, 1], f32, tag=f"bias{i}")
                nc.vector.tensor_tensor(
                    bias_i[:], at_sb[:, i : i + 1], b1_sb, ALU.add
                )
                ht_i = constp.tile([ANOTHER, S2], f32, tag=f"ht{i}")
                nc.scalar.activation(ht_i[:], hbt_sb[:], AF.Relu, bias=bias_i[:])
                th_ps = psA.tile([TOPICS, S2], f32, tag="ps")
                nc.tensor.matmul(th_ps[:], w2_sb, ht_i[:])
                et_i = constp.tile([TOPICS, S2], f32, tag=f"et{i}")
                nc.scalar.activation(et_i[:], th_ps[:], AF.Exp, bias=b2_sb)
                srow_ps = psA.tile([1, S2], f32, tag="ps")
                nc.tensor.matmul(srow_ps[:], ones_col[:], et_i[:])
                recip_i = constp.tile([1, S2], f32, tag=f"rc{i}")
                nc.vector.reciprocal(recip_i[:], srow_ps[:])
                bc_ps = psA.tile([TOPICS, S2], f32, tag="ps")
                nc.tensor.matmul(bc_ps[:], ones_row[0:1, 0:TOPICS], recip_i[:])
                pt_i = persist.tile([KE, S2], big_dt, tag=f"pt{i}")
                # ones row via SBUF->SBUF DMA (gpsimd casts f32->f32r; memset
                # can't target partition 50 / f32r directly); emitted before
                # the softmax TT so it overlaps the per-i chain
                nc.gpsimd.dma_start(pt_i[TOPICS:KE, :], ones_b[:])
                nc.vector.tensor_tensor(pt_i[0:TOPICS, :], et_i[:], bc_ps[:], ALU.mult)
                pt_ext.append(pt_i)

            # ---- S_ext = sum_chunks (pT chunk)^T-gram --------------------
            nchunk = SPC * S2 // 128  # 8
            pch = []
            for c in range(nchunk):
                i, rb = divmod(c, S2 // 128)
                tp_ps = psA.tile([128, KE], f32, tag="ps")
                nc.tensor.transpose(
                    tp_ps[:],
                    as_f32(pt_ext[i][:, rb * 128 : (rb + 1) * 128]),
                    ident[0:KE, 0:KE],
                )
                p_sb = constp.tile([128, KE], f32, tag=f"pch{c}")
                nc.vector.tensor_copy(p_sb[:], tp_ps[:])
                pch.append(p_sb)
            sext_ps = psA.tile([KE, KE], f32, tag="ps")
            for c in range(nchunk):
                nc.tensor.matmul(
                    sext_ps[:], pch[c][:], pch[c][:],
                    start=(c == 0), stop=(c == nchunk - 1),
                )
            sext_sb = constp.tile([KE, KE], f32)
            nc.vector.tensor_copy(sext_sb[:], sext_ps[:])

            # ---- AllReduce S_ext ----------------------------------------
            ar_in = dramp.tile([KE, KE], f32)
            ar_out = dramp.tile([KE, KE], f32)
            nc.sync.dma_start(ar_in[:], sext_sb[:])
            if TIMELINE_STUB:
                nc.sync.dma_start(ar_out[:], ar_in[:])
            else:
                nc.gpsimd.collective_compute(
                    "AllReduce", ALU.add,
                    replica_groups=[list(range(N_CORES))],
                    ins=[ar_in.opt()], outs=[ar_out.opt()],
                )
            sg = constp.tile([KE, KE], f32)
            nc.sync.dma_start(sg[:], ar_out[:])

            # ---- per-core vocab-slice stats ------------------------------
            inv_n = 1.0 / float(NTOT)
            pbar_n = constp.tile([TOPICS, 1], f32)
            nc.vector.tensor_scalar_mul(pbar_n[:], sg[0:TOPICS, TOPICS : TOPICS + 1], -inv_n)

            t_sb = constp.tile([TOPICS, VSL], f32)
            for h in range(2):
                sl = slice(h * 500, (h + 1) * 500)
                t_ps = psA.tile([TOPICS, 500], f32, tag="ps")
                nc.tensor.matmul(t_ps[:], sg[0:TOPICS, 0:TOPICS], w3sl_sb[:, sl])
                nc.vector.tensor_copy(t_sb[:, sl], t_ps[:])
            prod = constp.tile([TOPICS, VSL], f32)
            nc.vector.tensor_tensor(prod[:], t_sb[:], w3sl_sb, ALU.mult)
            # neg_m = -(u/n); BN is invariant to the b3 shift, so b3 never
            # appears: bias row = b3 - mean = neg_m, var = qd/n - (u/n)^2.
            neg_m = constp.tile([1, VSL], f32)
            msq = constp.tile([1, VSL], f32)
            m2 = constp.tile([1, VSL], f32)
            for h in range(2):
                sl = slice(h * 500, (h + 1) * 500)
                qd_ps = psA.tile([1, 500], f32, tag="ps")
                mean_ps = psA.tile([1, 500], f32, tag="ps")
                nc.tensor.matmul(qd_ps[:], ones_col[:], prod[:, sl])
                nc.tensor.matmul(mean_ps[:], pbar_n[:], w3sl_sb[:, sl])
                nc.vector.tensor_copy(neg_m[:, sl], mean_ps[:])
                nc.scalar.activation(m2[:, sl], mean_ps[:], AF.Square)
                nc.vector.tensor_scalar_mul(msq[:, sl], qd_ps[:], inv_n)
            # var+eps = (msq+eps)-m2, fused; rstd = exp(-0.5*ln(var+eps))
            var = constp.tile([1, VSL], f32)
            nc.vector.scalar_tensor_tensor(
                var[:], msq[:], BN_EPS, m2[:], ALU.add, ALU.subtract
            )
            lnv = constp.tile([1, VSL], f32)
            nc.scalar.activation(lnv[:], var[:], AF.Ln)
            rstd_sl = constp.tile([1, VSL], f32)
            nc.scalar.activation(rstd_sl[:], lnv[:], AF.Exp, scale=-0.5)
            biasr_sl = neg_m  # raw shift b3-mean == -(u/n)

            # ---- AllGather rstd/bias slices ------------------------------
            ag_in = dramp.tile([2, VSL], f32)
            ag_out = dramp.tile([2 * N_CORES, VSL], f32)
            nc.sync.dma_start(ag_in[0:1, :], rstd_sl[:])
            nc.sync.dma_start(ag_in[1:2, :], biasr_sl[:])
            if TIMELINE_STUB:
                for k in range(N_CORES):
                    nc.sync.dma_start(ag_out[2 * k : 2 * k + 2, :], ag_in[:])
            else:
                nc.gpsimd.collective_compute(
                    "AllGather", ALU.bypass,
                    replica_groups=[list(range(N_CORES))],
                    ins=[ag_in.opt()], outs=[ag_out.opt()],
                )
            # bias row: rows 1::2 of ag_out, one strided DMA
            ag_r = ag_out[:].rearrange("(k two) v -> two k v", two=2)
            nc.sync.dma_start(
                w3b[TOPICS:KE, :].rearrange("p (k v) -> p k v", k=N_CORES), ag_r[1]
            )
            # rstd broadcast to all 51 partitions straight from DRAM,
            # in 2-slice chunks so the W3s scaling can start early
            rbcast = constp.tile([KE, VOCAB], f32)
            for q in range(4):
                ksl = slice(2 * q, 2 * q + 2)
                nc.gpsimd.dma_start(
                    rbcast[:, 2 * q * VSL : (2 * q + 2) * VSL].rearrange(
                        "p (k v) -> p k v", k=2
                    ),
                    ag_r[0:1, ksl].to_broadcast([KE, 2, VSL]),
                )

            if DEBUG_OUT:
                nc.sync.dma_start(dbg_sg[:], sg[:])
                nc.sync.dma_start(dbg_stats[0:1, :], neg_m[:])
                nc.sync.dma_start(dbg_stats[1:2, :], var[:])
                nc.sync.dma_start(dbg_stats[2:3, :], rstd_sl[:])
                nc.sync.dma_start(dbg_stats[3:4, :], biasr_sl[:])

            # ---- W3s = W3b * rstd (chunked so the big stage can start early)
            for c in range(16):
                sl = slice(c * 500, (c + 1) * 500)
                nc.vector.tensor_tensor(w3s[:, sl], w3b[:, sl], rbcast[:, sl], ALU.mult)

            if DEBUG_OUT:
                nc.sync.dma_start(dbg_rstd[:], rbcast[0:1, :])
                nc.sync.dma_start(dbg_w3s[:], as_f32(w3s[:]))
                for i in range(SPC):
                    nc.sync.dma_start(
                        dbg_pt[:, i * S2 : (i + 1) * S2], as_f32(pt_ext[i][:])
                    )

        # ---- big stage: bn-matmul + exp + row-softmax -------------------
        with (
            tc.tile_pool(name="mm", bufs=2, space=bass.MemorySpace.PSUM) as mmp,
            tc.tile_pool(name="exp", bufs=3) as expp,
            tc.tile_pool(name="small", bufs=3) as smallp,
        ):
            nblk = RPC // 128  # 8
            for blk in range(nblk):
                i, rb = divmod(blk, S2 // 128)
                lhs = pt_ext[i][:, rb * 128 : (rb + 1) * 128]
                exp_t = expp.tile([128, VOCAB], f32, tag="exp")
                racc = smallp.tile([128, 4], f32, tag="racc")
                for g in range(4):
                    g0 = g * 2048
                    gw = min(2048, VOCAB - g0)  # 2048,2048,2048,1856
                    ps = mmp.tile([128, 2048], f32, tag="mm")
                    off = 0
                    while off < gw:
                        tw = min(512, gw - off)
                        nc.tensor.matmul(
                            ps[:, off : off + tw], lhs, w3s[:, g0 + off : g0 + off + tw],
                        )
                        off += tw
                    nc.scalar.activation(
                        exp_t[:, g0 : g0 + gw], ps[:, 0:gw], AF.Exp,
                        accum_out=racc[:, g : g + 1],
                    )
                rsum = smallp.tile([128, 1], f32, tag="rsum")
                nc.vector.reduce_sum(rsum[:], racc[:], axis=AX.X)
                rrec = smallp.tile([128, 1], f32, tag="rrec")
                nc.vector.reciprocal(rrec[:], rsum[:])
                if OUT16:
                    out_t = smallp.tile([128, VOCAB], out_dt, tag="out16")
                else:
                    out_t = exp_t
                for h in range(4):
                    sl = slice(h * 2000, (h + 1) * 2000)
                    nc.vector.tensor_scalar_mul(out_t[:, sl], exp_t[:, sl], rrec[:])
                    nc.sync.dma_start(
                        text_out[blk * 128 : (blk + 1) * 128, sl], out_t[:, sl]
                    )

    nc.compile()
    return nc


def _in_maps(zV, zW, bu, bp, bu_, bp_, W1, b1, W2, b2, W3, b3):
    f = lambda x: np.ascontiguousarray(np.asarray(x, dtype=np.float32))
    maps = []
    notes_lhsT = f(np.concatenate([zV.T, bu.T, bu_.T], axis=0))
    notes_rhs2 = f(np.concatenate([bp.T, bp_.T], axis=0))
    zwT = f(np.asarray(zW).T)
    common = {
        "zwT": zwT,
        "notes_lhsT": notes_lhsT,
        "notes_rhs2": notes_rhs2,
        "pack80": f(np.concatenate([np.asarray(W2), np.asarray(b1).reshape(ANOTHER, 1)], axis=1)),
        "w3": f(W3),
    }
    W1t = np.asarray(W1)[:INT_DIM]
    W1b = np.asarray(W1)[INT_DIM:]
    b2c = np.asarray(b2).reshape(TOPICS, 1)
    for k in range(N_CORES):
        m = dict(common)
        m["pack50"] = f(np.concatenate([
            W1t, W1b,
            np.asarray(zV)[k * SPC : (k + 1) * SPC].T,
            np.asarray(W3)[:, k * VSL : (k + 1) * VSL],
            b2c,
        ], axis=1))
        maps.append(m)
    return maps


LAST_RESULTS = None


def kernel(zV, zW, bu, bp, bu_, bp_, W1, b1, W2, b2, W3, b3):
    global LAST_RESULTS
    from concourse.bass_utils import run_bass_kernel_spmd

    key = "nc"
    if key not in _CACHE:
        _CACHE[key] = _build()
    nc = _CACHE[key]

    maps = _in_maps(zV, zW, bu, bp, bu_, bp_, W1, b1, W2, b2, W3, b3)
    res = run_bass_kernel_spmd(nc, maps, list(range(N_CORES)))
    LAST_RESULTS = res
    out_notes = np.asarray(res.results[0]["notes"], dtype=np.float32)
    out_text = np.concatenate(
        [np.asarray(res.results[k]["text_sh"]).astype(np.float32) for k in range(N_CORES)],
        axis=0,
    )
    return (out_notes, out_text)


# revision 26
# speedup vs baseline: 1.2398x; 1.0070x over previous
"""Trainium2 Bass kernel for the pairwise-decoder (dense_mlp) problem.

Computes, for s1=16, s2=512 grid of (i,j) pairs:
  out_notes = zV @ zW.T + bu @ bp.T + bu_ @ bp_.T                    [16, 512]
  feats[i,j] = [zV[i], zW[j]];  theta = relu(feats@W1+b1)@W2+b2
  p = softmax(theta); logits = p@W3+b3
  bn = (logits - mean0) * rsqrt(var0 + eps)   (batch stats over all 8192 rows)
  out_text = softmax(bn, axis=-1)                                    [8192, 8000]

Sharding: data-parallel over s1 (2 rows of zV per core -> 1024 grid rows/core).
BN batch stats use the Gram-matrix identity:
    sum_r logits[r,c]   = (pbar . w_c)          with pbar = sum_r p_r
    sum_r logits[r,c]^2 = w_c^T S w_c (+b3 terms), S = sum_r p_r p_r^T
so only S_ext = [p|1]^T [p|1]  (51x51) is all-reduced. Per-column stats math
is sharded over vocab (1000 cols/core via a host-sharded W3 slice), then
rstd / shift rows are all-gathered (b3 cancels exactly: BN output is
invariant to the b3 shift, and b3-mean = -(pbar@W3)/n). rstd and the shift
are folded into the weight matrix: W3s_ext = [W3*rstd ; -(u/n)*rstd],
pT_ext = [p^T ; 1], making the whole BN one K=51 float32r matmul; exp
row-sums come free from the scalar engine's accum_out, so the final softmax
costs one tensor_scalar multiply before the output DMA.
"""

import os
import numpy as np

# The trace path needs antenv.axon_hooks, absent in this container — make sure
# a stray BASS_TRACE in the environment can't break the grading run.
os.environ.setdefault("BASS_NEVER_TRACE", "1")

S1, S2 = 16, 512
INT_DIM, ANOTHER, TOPICS, VOCAB = 50, 80, 50, 8000
N_CORES = 8
SPC = S1 // N_CORES          # s1 rows per core (2)
RPC = SPC * S2               # grid rows per core (1024)
NTOT = S1 * S2               # 8192
VSL = VOCAB // N_CORES       # vocab stats slice per core (1000)
BN_EPS = 1e-5
KE = TOPICS + 1              # 51 (p rows + ones row)

_CACHE = {}

# env knobs for experiments
F32R_BIG = os.environ.get("NNK_F32R", "1") == "1"      # float32r for the big p@W3 matmul
OUT16 = os.environ.get("NNK_OUT16", "1") == "1"        # bf16 out_text DMA (host upcasts)
TIMELINE_STUB = os.environ.get("NNK_TIMELINE_STUB", "0") == "1"  # single-core, no collectives
DEBUG_OUT = os.environ.get("NNK_DEBUG", "0") == "1"  # extra intermediate outputs


def _build():
    import concourse.bass as bass
    import concourse.bacc as bacc
    import concourse.tile as tile
    import concourse.mybir as mybir
    from concourse.bass_utils import axon_active

    dt = mybir.dt
    f32 = dt.float32
    AF = mybir.ActivationFunctionType
    ALU = mybir.AluOpType
    AX = mybir.AxisListType

    nc = bacc.Bacc(
        "TRN2",
        target_bir_lowering=False,
        debug=False,
        enable_asserts=False,
        num_devices=1 if TIMELINE_STUB else N_CORES,
    )

    # ---- I/O --------------------------------------------------------------
    P50W = 2 * ANOTHER + SPC + VSL + 1  # w1t|w1b|zvT|w3sl|b2
    pack50 = nc.dram_tensor("pack50", [INT_DIM, P50W], f32, kind="ExternalInput")
    pack80 = nc.dram_tensor("pack80", [ANOTHER, TOPICS + 1], f32, kind="ExternalInput")
    zwT = nc.dram_tensor("zwT", [INT_DIM, S2], f32, kind="ExternalInput")
    notes_lhsT = nc.dram_tensor("notes_lhsT", [INT_DIM + 2, S1], f32, kind="ExternalInput")
    notes_rhs2 = nc.dram_tensor("notes_rhs2", [2, S2], f32, kind="ExternalInput")
    w3 = nc.dram_tensor("w3", [TOPICS, VOCAB], f32, kind="ExternalInput")

    notes_out = nc.dram_tensor("notes", [S1, S2], f32, kind="ExternalOutput")
    out_dt = dt.bfloat16 if OUT16 else f32
    text_out = nc.dram_tensor("text_sh", [RPC, VOCAB], out_dt, kind="ExternalOutput")
    if DEBUG_OUT:
        dbg_sg = nc.dram_tensor("dbg_sg", [KE, KE], f32, kind="ExternalOutput")
        dbg_rstd = nc.dram_tensor("dbg_rstd", [1, VOCAB], f32, kind="ExternalOutput")
        dbg_w3s = nc.dram_tensor("dbg_w3s", [KE, VOCAB], f32, kind="ExternalOutput")
        dbg_pt = nc.dram_tensor("dbg_pt", [KE, RPC], f32, kind="ExternalOutput")
        dbg_stats = nc.dram_tensor("dbg_stats", [4, VSL], f32, kind="ExternalOutput")

    ident_np = np.eye(128, dtype=np.float32)
    ident_dram = nc.inline_tensor(ident_np, name="ident128")

    big_dt = dt.float32r if F32R_BIG else f32

    def as_f32(ap):
        return ap.bitcast(f32) if big_dt != f32 else ap

    with tile.TileContext(nc) as tc:
      with tc.tile_pool(name="persist", bufs=1) as persist:
        with (
            tc.tile_pool(name="const", bufs=1) as constp,
            tc.tile_pool(name="psA", bufs=4, space=bass.MemorySpace.PSUM) as psA,
            tc.tile_pool(name="dram", bufs=1, space=bass.MemorySpace.DRAM) as dramp,
        ):
            # ---- constant loads ------------------------------------------
            ident = constp.tile([128, 128], f32)
            nc.sync.dma_start(ident[:], ident_dram[:])
            ones_row = constp.tile([1, KE], f32)       # [1,51] of ones
            nc.gpsimd.memset(ones_row[:], 1.0)
            ones_col = constp.tile([TOPICS, 1], f32)   # [50,1] of ones
            nc.gpsimd.memset(ones_col[:], 1.0)
            ones_b = constp.tile([1, S2], f32)         # [1,512] of ones
            nc.gpsimd.memset(ones_b[:], 1.0)

            p50_sb = constp.tile([INT_DIM, P50W], f32)
            nc.sync.dma_start(p50_sb[:], pack50[:])
            p80_sb = constp.tile([ANOTHER, TOPICS + 1], f32)
            nc.sync.dma_start(p80_sb[:], pack80[:])
            w1t_sb = p50_sb[:, 0:ANOTHER]
            w1b_sb = p50_sb[:, ANOTHER : 2 * ANOTHER]
            zvT_sb = p50_sb[:, 2 * ANOTHER : 2 * ANOTHER + SPC]
            w3sl_sb = p50_sb[:, 2 * ANOTHER + SPC : 2 * ANOTHER + SPC + VSL]
            b2_sb = p50_sb[:, P50W - 1 : P50W]
            w2_sb = p80_sb[:, 0:TOPICS]
            b1_sb = p80_sb[:, TOPICS : TOPICS + 1]

            # rhs for notes: [52,512] = [zW^T ; bp^T ; bp_^T]
            rhs_notes = constp.tile([INT_DIM + 2, S2], f32)
            nc.sync.dma_start(rhs_notes[0:INT_DIM, :], zwT[:])
            nc.sync.dma_start(rhs_notes[INT_DIM : INT_DIM + 2, :], notes_rhs2[:])
            lhsT_notes = constp.tile([INT_DIM + 2, S1], f32)
            nc.sync.dma_start(lhsT_notes[:], notes_lhsT[:])

            # big weights (row 50 = BN bias row, filled after AllGather)
            w3b = constp.tile([KE, VOCAB], f32)
            nc.sync.dma_start(w3b[0:TOPICS, :], w3[:])
            w3s = persist.tile([KE, VOCAB], big_dt)

            # ---- out_notes: one K=52 matmul ------------------------------
            notes_ps = psA.tile([S1, S2], f32, tag="ps")
            nc.tensor.matmul(notes_ps[:], lhsT_notes[:], rhs_notes[:])
            notes_sb = constp.tile([S1, S2], f32)
            nc.vector.tensor_copy(notes_sb[:], notes_ps[:])
            nc.sync.dma_start(notes_out[:], notes_sb[:])

            # ---- stage 1: p^T per i-block --------------------------------
            # hBT = W1_bot^T @ zW^T  (shared across i)
            hbt_ps = psA.tile([ANOTHER, S2], f32, tag="ps")
            nc.tensor.matmul(hbt_ps[:], w1b_sb, rhs_notes[0:INT_DIM, :])
            hbt_sb = constp.tile([ANOTHER, S2], f32)
            nc.vector.tensor_copy(hbt_sb[:], hbt_ps[:])
            # aT = W1_top^T @ zV^T : [80, SPC]
            at_ps = psA.tile([ANOTHER, SPC], f32, tag="ps")
            nc.tensor.matmul(at_ps[:], w1t_sb, zvT_sb)
            at_sb = constp.tile([ANOTHER, SPC], f32)
            nc.vector.tensor_copy(at_sb[:], at_ps[:])

            pt_ext = []
            for i in range(SPC):
                bias_i = constp.tile([ANOTHER, 1], f32, tag=f"bias{i}")
                nc.vector.tensor_tensor(
                    bias_i[:], at_sb[:, i : i + 1], b1_sb, ALU.add
                )
                ht_i = constp.tile([ANOTHER, S2], f32, tag=f"ht{i}")
                nc.scalar.activation(ht_i[:], hbt_sb[:], AF.Relu, bias=bias_i[:])
                th_ps = psA.tile([TOPICS, S2], f32, tag="ps")
                nc.tensor.matmul(th_ps[:], w2_sb, ht_i[:])
                et_i = constp.tile([TOPICS, S2], f32, tag=f"et{i}")
                nc.scalar.activation(et_i[:], th_ps[:], AF.Exp, bias=b2_sb)
                srow_ps = psA.tile([1, S2], f32, tag="ps")
                nc.tensor.matmul(srow_ps[:], ones_col[:], et_i[:])
                recip_i = constp.tile([1, S2], f32, tag=f"rc{i}")
                nc.vector.reciprocal(recip_i[:], srow_ps[:])
                bc_ps = psA.tile([TOPICS, S2], f32, tag="ps")
                nc.tensor.matmul(bc_ps[:], ones_row[0:1, 0:TOPICS], recip_i[:])
                pt_i = persist.tile([KE, S2], big_dt, tag=f"pt{i}")
                # ones row via SBUF->SBUF DMA (gpsimd casts f32->f32r; memset
                # can't target partition 50 / f32r directly); emitted before
                # the softmax TT so it overlaps the per-i chain
                nc.gpsimd.dma_start(pt_i[TOPICS:KE, :], ones_b[:])
                nc.vector.tensor_tensor(pt_i[0:TOPICS, :], et_i[:], bc_ps[:], ALU.mult)
                pt_ext.append(pt_i)

            # ---- S_ext = sum_chunks (pT chunk)^T-gram --------------------
            nchunk = SPC * S2 // 128  # 8
            pch = []
            for c in range(nchunk):
                i, rb = divmod(c, S2 // 128)
                tp_ps = psA.tile([128, KE], f32, tag="ps")
                nc.tensor.transpose(
                    tp_ps[:],
                    as_f32(pt_ext[i][:, rb * 128 : (rb + 1) * 128]),
                    ident[0:KE, 0:KE],
                )
                p_sb = constp.tile([128, KE], f32, tag=f"pch{c}")
                nc.vector.tensor_copy(p_sb[:], tp_ps[:])
                pch.append(p_sb)
            sext_ps = psA.tile([KE, KE], f32, tag="ps")
            for c in range(nchunk):
                nc.tensor.matmul(
                    sext_ps[:], pch[c][:], pch[c][:],
                    start=(c == 0), stop=(c == nchunk - 1),
                )
            sext_sb = constp.tile([KE, KE], f32)
            nc.vector.tensor_copy(sext_sb[:], sext_ps[:])

            # ---- AllReduce S_ext ----------------------------------------
            ar_in = dramp.tile([KE, KE], f32)
            ar_out = dramp.tile([KE, KE], f32)
            nc.sync.dma_start(ar_in[:], sext_sb[:])
            if TIMELINE_STUB:
                nc.sync.dma_start(ar_out[:], ar_in[:])
            else:
                nc.gpsimd.collective_compute(
                    "AllReduce", ALU.add,
                    replica_groups=[list(range(N_CORES))],
                    ins=[ar_in.opt()], outs=[ar_out.opt()],
                )
            sg = constp.tile([KE, KE], f32)
            nc.sync.dma_start(sg[:], ar_out[:])

            # ---- per-core vocab-slice stats ------------------------------
            inv_n = 1.0 / float(NTOT)
            pbar_n = constp.tile([TOPICS, 1], f32)
            nc.vector.tensor_scalar_mul(pbar_n[:], sg[0:TOPICS, TOPICS : TOPICS + 1], -inv_n)

            t_sb = constp.tile([TOPICS, VSL], f32)
            for h in range(2):
                sl = slice(h * 500, (h + 1) * 500)
                t_ps = psA.tile([TOPICS, 500], f32, tag="ps")
                nc.tensor.matmul(t_ps[:], sg[0:TOPICS, 0:TOPICS], w3sl_sb[:, sl])
                nc.vector.tensor_copy(t_sb[:, sl], t_ps[:])
            prod = constp.tile([TOPICS, VSL], f32)
            nc.vector.tensor_tensor(prod[:], t_sb[:], w3sl_sb, ALU.mult)
            # neg_m = -(u/n); BN is invariant to the b3 shift, so b3 never
            # appears: bias row = b3 - mean = neg_m, var = qd/n - (u/n)^2.
            neg_m = constp.tile([1, VSL], f32)
            msq = constp.tile([1, VSL], f32)
            m2 = constp.tile([1, VSL], f32)
            for h in range(2):
                sl = slice(h * 500, (h + 1) * 500)
                qd_ps = psA.tile([1, 500], f32, tag="ps")
                mean_ps = psA.tile([1, 500], f32, tag="ps")
                nc.tensor.matmul(qd_ps[:], ones_col[:], prod[:, sl])
                nc.tensor.matmul(mean_ps[:], pbar_n[:], w3sl_sb[:, sl])
                nc.vector.tensor_copy(neg_m[:, sl], mean_ps[:])
                nc.scalar.activation(m2[:, sl], mean_ps[:], AF.Square)
                nc.vector.tensor_scalar_mul(msq[:, sl], qd_ps[:], inv_n)
            # var+eps = (msq+eps)-m2, fused; rstd = exp(-0.5*ln(var+eps))
            var = constp.tile([1, VSL], f32)
            nc.vector.scalar_tensor_tensor(
                var[:], msq[:], BN_EPS, m2[:], ALU.add, ALU.subtract
            )
            lnv = constp.tile([1, VSL], f32)
            nc.scalar.activation(lnv[:], var[:], AF.Ln)
            rstd_sl = constp.tile([1, VSL], f32)
            nc.scalar.activation(rstd_sl[:], lnv[:], AF.Exp, scale=-0.5)
            biasr_sl = neg_m  # raw shift b3-mean == -(u/n)

            # ---- AllGather rstd/bias slices ------------------------------
            ag_in = dramp.tile([2, VSL], f32)
            ag_out = dramp.tile([2 * N_CORES, VSL], f32)
            nc.sync.dma_start(ag_in[0:1, :], rstd_sl[:])
            nc.sync.dma_start(ag_in[1:2, :], biasr_sl[:])
            if TIMELINE_STUB:
                for k in range(N_CORES):
                    nc.sync.dma_start(ag_out[2 * k : 2 * k + 2, :], ag_in[:])
            else:
                nc.gpsimd.collective_compute(
                    "AllGather", ALU.bypass,
                    replica_groups=[list(range(N_CORES))],
                    ins=[ag_in.opt()], outs=[ag_out.opt()],
                )
            # bias row: rows 1::2 of ag_out, one strided DMA
            ag_r = ag_out[:].rearrange("(k two) v -> two k v", two=2)
            nc.sync.dma_start(
                w3b[TOPICS:KE, :].rearrange("p (k v) -> p k v", k=N_CORES), ag_r[1]
            )
            # rstd broadcast to all 51 partitions straight from DRAM,
            # in 2-slice chunks so the W3s scaling can start early
            rbcast = constp.tile([KE, VOCAB], f32)
            for q in range(4):
                ksl = slice(2 * q, 2 * q + 2)
                nc.gpsimd.dma_start(
                    rbcast[:, 2 * q * VSL : (2 * q + 2) * VSL].rearrange(
                        "p (k v) -> p k v", k=2
                    ),
                    ag_r[0:1, ksl].to_broadcast([KE, 2, VSL]),
                )

            if DEBUG_OUT:
                nc.sync.dma_start(dbg_sg[:], sg[:])
                nc.sync.dma_start(dbg_stats[0:1, :], neg_m[:])
                nc.sync.dma_start(dbg_stats[1:2, :], var[:])
                nc.sync.dma_start(dbg_stats[2:3, :], rstd_sl[:])
                nc.sync.dma_start(dbg_stats[3:4, :], biasr_sl[:])

            # ---- W3s = W3b * rstd (chunked so the big stage can start early)
            for c in range(16):
                sl = slice(c * 500, (c + 1) * 500)
                nc.vector.tensor_tensor(w3s[:, sl], w3b[:, sl], rbcast[:, sl], ALU.mult)

            if DEBUG_OUT:
                nc.sync.dma_start(dbg_rstd[:], rbcast[0:1, :])
                nc.sync.dma_start(dbg_w3s[:], as_f32(w3s[:]))
                for i in range(SPC):
                    nc.sync.dma_start(
                        dbg_pt[:, i * S2 : (i + 1) * S2], as_f32(pt_ext[i][:])
                    )

        # ---- big stage: bn-matmul + exp + row-softmax -------------------
        with (
            tc.tile_pool(name="mm", bufs=2, space=bass.MemorySpace.PSUM) as mmp,
            tc.tile_pool(name="exp", bufs=3) as expp,
            tc.tile_pool(name="small", bufs=3) as smallp,
        ):
            nblk = RPC // 128  # 8
            for blk in range(nblk):
                i, rb = divmod(blk, S2 // 128)
                lhs = pt_ext[i][:, rb * 128 : (rb + 1) * 128]
                exp_t = expp.tile([128, VOCAB], f32, tag="exp")
                racc = smallp.tile([128, 4], f32, tag="racc")
                for g in range(4):
                    g0 = g * 2048
                    gw = min(2048, VOCAB - g0)  # 2048,2048,2048,1856
                    ps = mmp.tile([128, 2048], f32, tag="mm")
                    off = 0
                    while off < gw:
                        tw = min(512, gw - off)
                        nc.tensor.matmul(
                            ps[:, off : off + tw], lhs, w3s[:, g0 + off : g0 + off + tw],
                        )
                        off += tw
                    nc.scalar.activation(
                        exp_t[:, g0 : g0 + gw], ps[:, 0:gw], AF.Exp,
                        accum_out=racc[:, g : g + 1],
                    )
                rsum = smallp.tile([128, 1], f32, tag="rsum")
                nc.vector.reduce_sum(rsum[:], racc[:], axis=AX.X)
                rrec = smallp.tile([128, 1], f32, tag="rrec")
                nc.vector.reciprocal(rrec[:], rsum[:])
                if OUT16:
                    out_t = smallp.tile([128, VOCAB], out_dt, tag="out16")
                else:
                    out_t = exp_t
                for h in range(4):
                    sl = slice(h * 2000, (h + 1) * 2000)
                    nc.vector.tensor_scalar_mul(out_t[:, sl], exp_t[:, sl], rrec[:])
                    nc.sync.dma_start(
                        text_out[blk * 128 : (blk + 1) * 128, sl], out_t[:, sl]
                    )

    nc.compile()
    return nc


def _in_maps(zV, zW, bu, bp, bu_, bp_, W1, b1, W2, b2, W3, b3):
    f = lambda x: np.ascontiguousarray(np.asarray(x, dtype=np.float32))
    maps = []
    notes_lhsT = f(np.concatenate([zV.T, bu.T, bu_.T], axis=0))
    notes_rhs2 = f(np.concatenate([bp.T, bp_.T], axis=0))
    zwT = f(np.asarray(zW).T)
    common = {
        "zwT": zwT,
        "notes_lhsT": notes_lhsT,
        "notes_rhs2": notes_rhs2,
        "pack80": f(np.concatenate([np.asarray(W2), np.asarray(b1).reshape(ANOTHER, 1)], axis=1)),
        "w3": f(W3),
    }
    W1t = np.asarray(W1)[:INT_DIM]
    W1b = np.asarray(W1)[INT_DIM:]
    b2c = np.asarray(b2).reshape(TOPICS, 1)
    for k in range(N_CORES):
        m = dict(common)
        m["pack50"] = f(np.concatenate([
            W1t, W1b,
            np.asarray(zV)[k * SPC : (k + 1) * SPC].T,
            np.asarray(W3)[:, k * VSL : (k + 1) * VSL],
            b2c,
        ], axis=1))
        maps.append(m)
    return maps


LAST_RESULTS = None


def kernel(zV, zW, bu, bp, bu_, bp_, W1, b1, W2, b2, W3, b3):
    global LAST_RESULTS
    from concourse.bass_utils import run_bass_kernel_spmd

    key = "nc"
    if key not in _CACHE:
        _CACHE[key] = _build()
    nc = _CACHE[key]

    maps = _in_maps(zV, zW, bu, bp, bu_, bp_, W1, b1, W2, b2, W3, b3)
    res = run_bass_kernel_spmd(nc, maps, list(range(N_CORES)))
    LAST_RESULTS = res
    out_notes = np.asarray(res.results[0]["notes"], dtype=np.float32)
    out_text = np.concatenate(
        [np.asarray(res.results[k]["text_sh"]).astype(np.float32) for k in range(N_CORES)],
        axis=0,
    )
    return (out_notes, out_text)
